# revision 1
# baseline (speedup 1.0000x reference)
"""Trainium2 Bass kernel for the ActionLayer top-k masking loss.

Computation per token (vocab V=32000, hidden H=1024):
  - top-16 logits (+ label, deduped) form the action set
  - gather action embeddings, run [hidden|embed] -> 2H -> GELU -> H MLP, LayerNorm
  - cosine(action_repr, future) -> softmax rewards
  - loss = -sum(rewards * log_softmax(logits)[actions]) / |actions|, averaged over
    valid tokens.

Sharding: data-parallel over the 2048 flattened tokens across 8 NeuronCores
(256 tokens/core); embed/MLP weights replicated; scalar loss reduced on host.

Device-side plan per core (tokens-on-partitions for the vocab phase,
feature-dims-on-partitions for the MLP phase; action columns ordered
i = slot*256 + token):
  Stage A (per 128-token tile): stream logits; segmented max over groups of
    256 -> top-16 groups (DVE max/max_index/match_replace); dma_gather the 16
    winner groups/token (<=1024 idxs per call; gpsimd gathers are ordered to
    never overlap ACT activity, which wedges the device); top-16 of the
    gathered 4096 -> exact values + positions; group-id one-hot select gives
    global indices; fused exp+accumulate for logsumexp.
  Stage B: pre-gather all action embeddings transposed (dma_gather
    transpose=True) into a DRAM bounce buffer during an ACT-free window, then
    W1/W2 matmuls with weights stationary, GELU(+b1) on ACT, LayerNorm via
    ones-matmul column sums, cosine scores vs future.
  Stage C: per-token softmax over 17 action slots, combine with
    (logit - logsumexp), reduce to per-core loss numerator + valid count.
"""

import numpy as np
import ml_dtypes

BF16 = ml_dtypes.bfloat16
NEG_BIG = -3.0e38


# ---------------------------------------------------------------------------
# device program
# ---------------------------------------------------------------------------

def build_program(V=32000, TPC=256, H=1024, INNER=2048, GRP=256, debug=False,
                  stages="ABC", no_label=False, no_grp=False, no_emb=False,
                  no_exp=False, dbg_dump=False):
    """Build the per-core Bass program. Returns the compiled Bass object."""
    import concourse.bacc as bacc
    import concourse.mybir as mybir
    import concourse.tile as tile
    from concourse.bass import IndirectOffsetOnAxis
    from concourse.tile import add_dep_helper

    dt = mybir.dt
    Alu = mybir.AluOpType
    Act = mybir.ActivationFunctionType
    Ax = mybir.AxisListType

    TT = TPC // 128              # token tiles per core
    G = V // GRP                 # groups per token row
    K16 = 16                     # top-k
    A = 17                       # action slots (16 topk + label)
    NCOLS = A * TPC              # action columns
    NW = NCOLS // 16             # wrapped idx width
    KH = H // 128                # 8  k-tiles for H-contraction
    MI = INNER // 128            # 16 m-tiles of inner dim
    MH = H // 128                # 8  m-tiles of H output
    SUB = 5                      # logits sub-tiles per token tile
    VSUB = V // SUB              # sub-tile width
    GS = G // SUB                # groups per sub-tile
    assert V % (SUB * GRP) == 0 and TPC % 128 == 0 and H % 128 == 0
    assert (TPC - 1) * G + G - 1 < 32768  # gather row ids must fit int16

    nc = bacc.Bacc("TRN2", target_bir_lowering=False, debug=debug, num_devices=8)

    # ---- DRAM I/O ----
    logits = nc.dram_tensor("logits", [TPC, V], dt.float32, kind="ExternalInput")
    hiddenT = nc.dram_tensor("hiddenT", [H, TPC], dt.bfloat16, kind="ExternalInput")
    futT = nc.dram_tensor("futT", [H, TPC], dt.bfloat16, kind="ExternalInput")
    embed = nc.dram_tensor("embed", [V, H], dt.bfloat16, kind="ExternalInput")
    W1 = nc.dram_tensor("W1", [INNER, INNER], dt.bfloat16, kind="ExternalInput")
    W2 = nc.dram_tensor("W2", [INNER, H], dt.bfloat16, kind="ExternalInput")
    w2r = nc.dram_tensor("w2r", [128, MI], dt.bfloat16, kind="ExternalInput")
    b1d = nc.dram_tensor("b1d", [128, MI], dt.float32, kind="ExternalInput")
    b2d = nc.dram_tensor("b2d", [128, MH], dt.float32, kind="ExternalInput")
    gd = nc.dram_tensor("gd", [128, MH], dt.float32, kind="ExternalInput")
    bd = nc.dram_tensor("bd", [128, MH], dt.float32, kind="ExternalInput")
    sumb2 = nc.dram_tensor("sumb2", [1, 1], dt.float32, kind="ExternalInput")
    labf = nc.dram_tensor("labf", [128, TT], dt.float32, kind="ExternalInput")
    fvf = nc.dram_tensor("fvf", [128, TT], dt.float32, kind="ExternalInput")
    lgidx = nc.dram_tensor("lgidx", [128, TT], dt.uint32, kind="ExternalInput")
    lwrap = nc.dram_tensor("lwrap", [16, NW], dt.int16, kind="ExternalInput")
    jio = nc.dram_tensor("jio", [128, 16], dt.float32, kind="ExternalInput")
    pfG = nc.dram_tensor("pfG", [128, 1], dt.float32, kind="ExternalInput")
    thr = nc.dram_tensor("thr", [128, 15], dt.float32, kind="ExternalInput")
    onesbf = nc.dram_tensor("onesbf", [128, 1], dt.bfloat16, kind="ExternalInput")
    onesrow = nc.dram_tensor("onesrow", [1, 128], dt.float32, kind="ExternalInput")
    onescol = nc.dram_tensor("onescol", [128, 1], dt.float32, kind="ExternalInput")
    outd = nc.dram_tensor("out", [1, 8], dt.float32, kind="ExternalOutput")
    if dbg_dump:
        dbg_gidf = nc.dram_tensor("dbg_gidf", [128, TT * K16], dt.float32,
                                  kind="ExternalOutput")
        dbg_idx = nc.dram_tensor("dbg_idx", [128, TT * K16], dt.int16,
                                 kind="ExternalOutput")
        dbg_wrgf = nc.dram_tensor("dbg_wrgf", [128, TT * 128], dt.int16,
                                  kind="ExternalOutput")
    scd = nc.dram_tensor("scd", [NCOLS], dt.float32)
    etd = nc.dram_tensor("etd", [128, KH, NCOLS], dt.bfloat16)

    f32, bf16 = dt.float32, dt.bfloat16

    with tile.TileContext(nc) as tc:
        # ---------- persistent small tiles ----------
        with tc.tile_pool(name="persist", bufs=1) as pp:
            labsb = pp.tile([128, TT], f32, tag="labsb")
            fvsb = pp.tile([128, TT], f32, tag="fvsb")
            lgisb = pp.tile([128, TT], dt.uint32, tag="lgisb")
            jiosb = pp.tile([128, K16], f32, tag="jiosb")
            pfGsb = pp.tile([128, 1], f32, tag="pfGsb")
            thrsb = pp.tile([128, 15], f32, tag="thrsb")
            wr16 = pp.tile([16, NW], dt.int16, tag="wr16")
            vals17 = pp.tile([128, TT, A], f32, tag="vals17")
            glob = pp.tile([128, TT, K16], f32, tag="glob")
            dup = pp.tile([128, TT], f32, tag="dup")
            lse = pp.tile([128, TT], f32, tag="lse")
            accs = pp.tile([128, SUB * TT], f32, tag="accs")
            Ssum = pp.tile([128, TT], f32, tag="Ssum")

            nc.sync.dma_start(labsb[:], labf.ap())
            nc.sync.dma_start(fvsb[:], fvf.ap())
            nc.sync.dma_start(lgisb[:], lgidx.ap())
            nc.sync.dma_start(jiosb[:], jio.ap())
            nc.sync.dma_start(pfGsb[:], pfG.ap())
            nc.sync.dma_start(thrsb[:], thr.ap())
            nc.sync.dma_start(wr16[:], lwrap.ap())

            # ================= Stage A: topk + logsumexp =================
            # ACT-exp concurrent with gpsimd dma_gather wedges the device;
            # collect insts and serialize gathers after all exps.
            exp_insts = []
            gather_insts = []
            with tc.tile_pool(name="stageA", bufs=1) as pa:
                for tt in range(TT):
                    M = pa.tile([128, G], f32, tag="M", bufs=2)
                    for h in range(SUB):
                        lg = pa.tile([128, VSUB], f32, tag="lg", bufs=2)
                        nc.sync.dma_start(
                            lg[:], logits.ap()[tt * 128:(tt + 1) * 128,
                                               h * VSUB:(h + 1) * VSUB])
                        # segmented max -> group maxes
                        nc.vector.tensor_reduce(
                            M[:, h * GS:(h + 1) * GS],
                            lg.rearrange("p (g e) -> p g e", e=GRP),
                            axis=Ax.X, op=Alu.max)
                        # fused exp + accumulate (logits ~ N(0,1): no max-shift
                        # needed for fp32 exp-sum stability)
                        if no_exp:
                            nc.vector.memset(
                                accs[:, SUB * tt + h:SUB * tt + h + 1], 1.0)
                        else:
                            esc = pa.tile([128, VSUB], bf16, tag="esc", bufs=1)
                            ei = nc.scalar.activation(
                                esc[:], lg[:], Act.Exp,
                                accum_out=accs[:, SUB * tt + h:SUB * tt + h + 1])
                            exp_insts.append(ei)

                    # top-16 groups of M
                    m8a = pa.tile([128, 8], f32, tag="m8a", bufs=2)
                    m8b = pa.tile([128, 8], f32, tag="m8b", bufs=2)
                    gidu = pa.tile([128, K16], dt.uint32, tag="gidu", bufs=2)
                    Mx = pa.tile([128, G], f32, tag="Mx", bufs=2)
                    nc.vector.max(m8a[:], M[:])
                    nc.vector.max_index(gidu[:, 0:8], m8a[:], M[:])
                    nc.vector.match_replace(Mx[:], m8a[:], M[:], NEG_BIG)
                    nc.vector.max(m8b[:], Mx[:])
                    nc.vector.max_index(gidu[:, 8:16], m8b[:], Mx[:])

                    gidf = pa.tile([128, K16], f32, tag="gidf", bufs=2)
                    nc.vector.tensor_copy(gidf[:], gidu[:])

                    # winner-group gather indices: row = p*G + gid  (int16-safe)
                    idxg = pa.tile([128, K16], f32, tag="idxg", bufs=2)
                    nc.vector.tensor_scalar(idxg[:], gidf[:], pfGsb[:], None,
                                            op0=Alu.add)
                    if tt:
                        nc.vector.tensor_scalar(idxg[:], idxg[:],
                                                float(tt * 128 * G), None,
                                                op0=Alu.add)
                    idxg16 = pa.tile([128, K16], dt.int16, tag="idxg16", bufs=2)
                    nc.vector.tensor_copy(idxg16[:], idxg[:])
                    wrg = pa.tile([16, 8 * K16], dt.int16, tag="wrg", bufs=2)
                    wrgv = wrg.rearrange("q (j u) -> q j u", u=8)
                    ig16v = idxg16  # [128, 16]
                    for u in range(8):
                        nc.sync.dma_start(wrgv[:, :, u],
                                          ig16v[16 * u:16 * (u + 1), :])
                    wrgf = pa.tile([128, 8 * K16], dt.int16, tag="wrgf", bufs=2)
                    for r in range(8):
                        nc.sync.dma_start(wrgf[16 * r:16 * (r + 1), :], wrg[:])

                    # gather the 16 winner groups per token from DRAM logits
                    if dbg_dump:
                        nc.sync.dma_start(
                            dbg_gidf.ap()[:, tt * K16:(tt + 1) * K16], gidf[:])
                        nc.sync.dma_start(
                            dbg_idx.ap()[:, tt * K16:(tt + 1) * K16], idxg16[:])
                        nc.sync.dma_start(
                            dbg_wrgf.ap()[:, tt * 128:(tt + 1) * 128], wrgf[:])
                    grp = pa.tile([128, K16, GRP], f32, tag="grp", bufs=2)
                    if no_grp:
                        nc.vector.memset(grp[:], 0.0)
                    else:
                        # HW limit: dma_gather handles <=1024 idxs per call
                        lgv = logits.ap().rearrange("t (g e) -> (t g) e", e=GRP)
                        for kk in range(2):
                            gi = nc.gpsimd.dma_gather(
                                out_ap=grp[:, 8 * kk:8 * (kk + 1), :],
                                in_ap=lgv,
                                idxs_ap=wrgf[:, 64 * kk:64 * (kk + 1)],
                                num_idxs=1024, num_idxs_reg=1024,
                                elem_size=GRP)
                            gather_insts.append(gi)

                    # exact top-16 of the gathered candidates
                    grpv = grp.rearrange("p j e -> p (j e)")
                    pos = pa.tile([128, K16], dt.uint32, tag="pos", bufs=2)
                    nc.vector.max(vals17[:, tt, 0:8], grpv)
                    nc.vector.max_index(pos[:, 0:8], vals17[:, tt, 0:8], grpv)
                    nc.vector.match_replace(grpv, vals17[:, tt, 0:8], grpv, NEG_BIG)
                    nc.vector.max(vals17[:, tt, 8:16], grpv)
                    nc.vector.max_index(pos[:, 8:16], vals17[:, tt, 8:16], grpv)

                    # positions -> (group slot j, offset e); global = gid[j]*GRP + e
                    posf = pa.tile([128, K16], f32, tag="posf", bufs=2)
                    nc.vector.tensor_copy(posf[:], pos[:])
                    cmp = pa.tile([128, K16, 15], f32, tag="cmp", bufs=2)
                    nc.vector.tensor_tensor(
                        cmp[:],
                        posf.unsqueeze(2).to_broadcast([128, K16, 15]),
                        thrsb.unsqueeze(1).to_broadcast([128, K16, 15]),
                        op=Alu.is_ge)
                    jf = pa.tile([128, K16], f32, tag="jf", bufs=2)
                    nc.vector.tensor_reduce(jf[:], cmp[:], axis=Ax.X, op=Alu.add)
                    ef = pa.tile([128, K16], f32, tag="ef", bufs=2)
                    nc.vector.tensor_scalar(ef[:], jf[:], -float(GRP), None,
                                            op0=Alu.mult)
                    nc.vector.tensor_add(ef[:], ef[:], posf[:])

                    # group-id select: gsel[p,k] = sum_j (jf[p,k]==j) * gid[p,j]
                    cmp2 = pa.tile([128, K16, K16], f32, tag="cmp2", bufs=2)
                    nc.vector.tensor_tensor(
                        cmp2[:],
                        jf.unsqueeze(2).to_broadcast([128, K16, K16]),
                        jiosb.unsqueeze(1).to_broadcast([128, K16, K16]),
                        op=Alu.is_equal)
                    nc.vector.tensor_tensor(
                        cmp2[:], cmp2[:],
                        gidf.unsqueeze(1).to_broadcast([128, K16, K16]),
                        op=Alu.mult)
                    gsel = pa.tile([128, K16], f32, tag="gsel", bufs=2)
                    nc.vector.tensor_reduce(gsel[:], cmp2[:], axis=Ax.X, op=Alu.add)
                    nc.vector.tensor_scalar(glob[:, tt, :], gsel[:], float(GRP),
                                            None, op0=Alu.mult)
                    nc.vector.tensor_add(glob[:, tt, :], glob[:, tt, :], ef[:])

                    # label logit gather (per-partition indirect)
                    if no_label:
                        nc.vector.memset(vals17[:, tt, 16:17], 0.0)
                    else:
                        nc.gpsimd.indirect_dma_start(
                            out=vals17[:, tt, 16:17], out_offset=None,
                            in_=logits.ap().rearrange("t v -> (t v)").unsqueeze(1),
                            in_offset=IndirectOffsetOnAxis(ap=lgisb[:, tt:tt + 1], axis=0))

                    # is_dup = any(topk == label)
                    eqs = pa.tile([128, K16], f32, tag="eqs", bufs=2)
                    nc.vector.tensor_scalar(eqs[:], glob[:, tt, :],
                                            labsb[:, tt:tt + 1], None,
                                            op0=Alu.is_equal)
                    nc.vector.tensor_reduce(dup[:, tt:tt + 1], eqs[:], axis=Ax.X,
                                            op=Alu.max)

                    # embed-gather wrapped idx (slots 0..15; label slot from template)
                    gl16 = pa.tile([128, K16], dt.int16, tag="gl16", bufs=2)
                    nc.vector.tensor_copy(gl16[:], glob[:, tt, :])
                    wr16v = wr16.rearrange("q (a u) -> q a u", u=NW // A)
                    for u in range(8):
                        nc.sync.dma_start(wr16v[:, 0:K16, tt * 8 + u],
                                          gl16[16 * u:16 * (u + 1), :])

                # logsumexp = ln(sum(exp))
                for tt in range(TT):
                    nc.vector.tensor_reduce(
                        Ssum[:, tt:tt + 1], accs[:, SUB * tt:SUB * (tt + 1)],
                        axis=Ax.X, op=Alu.add)
                li = nc.scalar.activation(lse[:], Ssum[:], Act.Ln)
                for g_ in gather_insts:
                    for e_ in exp_insts:
                        add_dep_helper(g_.ins, e_.ins, sync=True,
                                       reason="serialize gather after exp")
                    add_dep_helper(li.ins, g_.ins, sync=True,
                                   reason="serialize lse after gather")

            # ================= Stage B: embeds + MLP + scores ============
            CH = 512
            NCH = (NCOLS + CH - 1) // CH
            if "B" not in stages:
                with tc.tile_pool(name="dummy", bufs=1) as pdum:
                    dz = pdum.tile([1, 8], f32, tag="dz")
                    nc.vector.memset(dz[:], 0.0)
                    nc.vector.tensor_add(dz[:, 0:1], vals17[0:1, 0, 0:1],
                                         glob[0:1, 0, 0:1])
                    nc.sync.dma_start(outd.ap(), dz[:])
            else:
              with tc.tile_pool(name="stageB", bufs=1) as pb, \
                   tc.tile_pool(name="psum", bufs=1, space="PSUM") as ps:
                    W1es = pb.tile([128, KH, INNER], bf16, tag="W1es")
                    W2s = pb.tile([128, MI, H], bf16, tag="W2s")
                    w2rs = pb.tile([128, MI], bf16, tag="w2rs")
                    hTsb = pb.tile([128, KH, TPC], bf16, tag="hTsb")
                    fTsb = pb.tile([128, KH, TPC], bf16, tag="fTsb")
                    b1s = pb.tile([128, MI], f32, tag="b1s")
                    b2s = pb.tile([128, MH], f32, tag="b2s")
                    gs = pb.tile([128, MH], f32, tag="gs")
                    bs = pb.tile([128, MH], f32, tag="bs")
                    sb2 = pb.tile([1, 1], f32, tag="sb2")
                    epsb = pb.tile([1, 1], f32, tag="epsb")
                    nc.vector.memset(epsb[:], 1e-5)
                    obf = pb.tile([128, 1], bf16, tag="obf")
                    orow = pb.tile([1, 128], f32, tag="orow")
                    ocol = pb.tile([128, 1], f32, tag="ocol")
                    wrf = pb.tile([128, NW], dt.int16, tag="wrf")

                    nc.sync.dma_start(
                        W1es[:], W1.ap()[H:INNER, :].rearrange("(g p) j -> p g j", p=128))
                    nc.sync.dma_start(W2s[:], W2.ap().rearrange("(g p) j -> p g j", p=128))
                    nc.sync.dma_start(w2rs[:], w2r.ap())
                    nc.sync.dma_start(hTsb[:], hiddenT.ap().rearrange("(g p) t -> p g t", p=128))
                    nc.sync.dma_start(fTsb[:], futT.ap().rearrange("(g p) t -> p g t", p=128))
                    nc.sync.dma_start(b1s[:], b1d.ap())
                    nc.sync.dma_start(b2s[:], b2d.ap())
                    nc.sync.dma_start(gs[:], gd.ap())
                    nc.sync.dma_start(bs[:], bd.ap())
                    nc.sync.dma_start(sb2[:], sumb2.ap())
                    nc.sync.dma_start(obf[:], onesbf.ap())
                    nc.sync.dma_start(orow[:], onesrow.ap())
                    nc.sync.dma_start(ocol[:], onescol.ap())
                    for r in range(8):
                        nc.sync.dma_start(wrf[16 * r:16 * (r + 1), :], wr16[:])

                    # Pre-gather ALL action embeddings to DRAM: gpsimd
                    # dma_gather concurrent with ACT activity wedges the
                    # device, so run every gather after stage-A's last ACT op
                    # and before any stage-B ACT op.
                    emb_gathers = []
                    for c in range(NCH):
                        W = min(CH, NCOLS - c * CH)
                        ETg = pb.tile([128, KH, W], bf16, tag="ET", bufs=2)
                        if no_emb:
                            nc.vector.memset(ETg[:], 0.0)
                        else:
                            gi = nc.gpsimd.dma_gather(
                                out_ap=ETg[:], in_ap=embed.ap(),
                                idxs_ap=wrf[:, c * (CH // 16):
                                            c * (CH // 16) + W // 16],
                                num_idxs=W, num_idxs_reg=W, elem_size=H,
                                transpose=True)
                            add_dep_helper(gi.ins, li.ins, sync=True,
                                           reason="gather after stage-A ACT")
                            emb_gathers.append(gi)
                        nc.sync.dma_start(etd.ap()[:, :, c * CH:c * CH + W],
                                          ETg[:])

                    # future squared norms -> s2f row [1, TPC]
                    fsq = pb.tile([128, KH, TPC], bf16, tag="ET", bufs=2)
                    nc.vector.tensor_mul(fsq[:], fTsb[:], fTsb[:])
                    ps2f = ps.tile([1, TPC], f32, tag="psrow", bufs=2)
                    for k in range(KH):
                        nc.tensor.matmul(ps2f[:], obf[:], fsq[:, k, :],
                                         start=(k == 0), stop=(k == KH - 1))
                    s2f = pb.tile([1, TPC], f32, tag="s2f")
                    s2fi = nc.scalar.copy(s2f[:], ps2f[:])
                    for g_ in emb_gathers:
                        add_dep_helper(s2fi.ins, g_.ins, sync=True,
                                       reason="ACT after emb gathers")

                    # hidden half of W1 (per token, broadcast later over slots)
                    hTh = pb.tile([128, MI, TPC], bf16, tag="hTh")
                    with tc.tile_pool(name="w1h", bufs=1) as pw:
                        W1hs = pw.tile([128, KH, INNER], bf16, tag="W1hs")
                        nc.sync.dma_start(
                            W1hs[:], W1.ap()[0:H, :].rearrange("(g p) j -> p g j", p=128))
                        for m in range(MI):
                            psh = ps.tile([128, TPC], f32, tag="psA", bufs=2)
                            for k in range(KH):
                                nc.tensor.matmul(psh[:],
                                                 W1hs[:, k, 128 * m:128 * (m + 1)],
                                                 hTsb[:, k, :],
                                                 start=(k == 0), stop=(k == KH - 1))
                            hi = nc.scalar.copy(hTh[:, m, :], psh[:])
                            for g_ in emb_gathers:
                                add_dep_helper(hi.ins, g_.ins, sync=True,
                                               reason="ACT after emb gathers")

                    for c in range(NCH):
                        W = min(CH, NCOLS - c * CH)
                        NS = W // TPC  # slots in this chunk
                        # gather action embeddings, transposed: [128, KH, W]
                        ET = pb.tile([128, KH, W], bf16, tag="ET", bufs=2)
                        nc.sync.dma_start(ET[:],
                                          etd.ap()[:, :, c * CH:c * CH + W])

                        # ---- W1 embed half + GELU ----
                        hTc = pb.tile([128, MI, W], bf16, tag="hTc", bufs=2)
                        for m in range(MI):
                            psA = ps.tile([128, W], f32, tag="psA", bufs=2)
                            for k in range(KH):
                                nc.tensor.matmul(
                                    psA[:],
                                    W1es[:, k, 128 * m:128 * (m + 1)],
                                    ET[:, k, :],
                                    start=(k == 0), stop=(k == KH - 1))
                            hpre = pb.tile([128, W], bf16, tag="hpre", bufs=2)
                            hb = hTh[:, m, :].unsqueeze(1).to_broadcast([128, NS, TPC])
                            nc.vector.tensor_tensor(
                                hpre.rearrange("p (s t) -> p s t", t=TPC),
                                psA.rearrange("p (s t) -> p s t", t=TPC),
                                hb, op=Alu.add)
                            nc.scalar.activation(hTc[:, m, :], hpre[:], Act.Gelu,
                                                 bias=b1s[:, m:m + 1])

                        # ---- W2 + column sums ----
                        psS = ps.tile([1, W], f32, tag="psrow", bufs=2)
                        dT = pb.tile([128, MH, W], bf16, tag="dT", bufs=2)
                        psQ = ps.tile([1, W], f32, tag="psrow", bufs=2)
                        for mh in range(MH):
                            psB = ps.tile([128, W], f32, tag="psB", bufs=2)
                            for k in range(MI):
                                nc.tensor.matmul(psB[:],
                                                 W2s[:, k, 128 * mh:128 * (mh + 1)],
                                                 hTc[:, k, :],
                                                 start=(k == 0), stop=(k == MI - 1))
                            # delta + b2 (bf16) and its square
                            nc.scalar.activation(dT[:, mh, :], psB[:],
                                                 Act.Identity, bias=b2s[:, mh:mh + 1])
                            nc.tensor.matmul(psS[:], obf[:], dT[:, mh, :],
                                             start=(mh == 0), stop=(mh == MH - 1))
                            sq = pb.tile([128, W], bf16, tag="sq", bufs=2)
                            nc.scalar.activation(sq[:], psB[:], Act.Square,
                                                 bias=b2s[:, mh:mh + 1])
                            nc.tensor.matmul(psQ[:], obf[:], sq[:],
                                             start=(mh == 0), stop=(mh == MH - 1))

                        # ---- LayerNorm rows ----
                        mur = pb.tile([1, W], f32, tag="mur", bufs=1)
                        nc.vector.tensor_scalar(mur[:], psS[:],
                                                1.0 / H, sb2[:], op0=Alu.mult,
                                                op1=Alu.add)
                        varr = pb.tile([1, W], f32, tag="varr", bufs=1)
                        nc.vector.tensor_scalar(varr[:], psQ[:], 1.0 / H,
                                                None, op0=Alu.mult)
                        mu2 = pb.tile([1, W], f32, tag="mu2", bufs=1)
                        nc.vector.tensor_mul(mu2[:], mur[:], mur[:])
                        nc.vector.tensor_sub(varr[:], varr[:], mu2[:])
                        sroot = pb.tile([1, W], f32, tag="sroot", bufs=1)
                        nc.scalar.activation(sroot[:], varr[:], Act.Sqrt,
                                             bias=epsb[:])
                        rstd = pb.tile([1, W], f32, tag="rstd", bufs=1)
                        nc.vector.reciprocal(rstd[:], sroot[:])
                        brow = pb.tile([1, W], f32, tag="brow", bufs=1)
                        nc.vector.tensor_mul(brow[:], mur[:], rstd[:])

                        # broadcast rstd/brow across partitions via K=1 matmul
                        psR = ps.tile([128, W], f32, tag="psbc", bufs=2)
                        nc.tensor.matmul(psR[:], orow[:], rstd[:])
                        rstdb = pb.tile([128, W], bf16, tag="rstdb", bufs=1)
                        nc.scalar.copy(rstdb[:], psR[:])
                        psM = ps.tile([128, W], f32, tag="psbc", bufs=2)
                        nc.tensor.matmul(psM[:], orow[:], brow[:])
                        browb = pb.tile([128, W], bf16, tag="browb", bufs=1)
                        nc.scalar.copy(browb[:], psM[:])

                        # ---- normalize + cosine numerator/denominator ----
                        psS2 = ps.tile([1, W], f32, tag="psrow", bufs=2)
                        psD = ps.tile([1, W], f32, tag="psrow", bufs=2)
                        for mh in range(MH):
                            nr = pb.tile([128, W], bf16, tag="nr", bufs=2)
                            nc.vector.tensor_mul(nr[:], dT[:, mh, :], rstdb[:])
                            nc.vector.tensor_sub(nr[:], nr[:], browb[:])
                            nc.vector.tensor_scalar(nr[:], nr[:],
                                                    gs[:, mh:mh + 1], bs[:, mh:mh + 1],
                                                    op0=Alu.mult, op1=Alu.add)
                            sqn = pb.tile([128, W], bf16, tag="sqn", bufs=2)
                            nc.vector.tensor_mul(sqn[:], nr[:], nr[:])
                            nc.tensor.matmul(psS2[:], obf[:], sqn[:],
                                             start=(mh == 0), stop=(mh == MH - 1))
                            pr = pb.tile([128, W], bf16, tag="pr", bufs=2)
                            fb = fTsb[:, mh, :].unsqueeze(1).to_broadcast([128, NS, TPC])
                            nc.vector.tensor_tensor(
                                pr.rearrange("p (s t) -> p s t", t=TPC),
                                nr.rearrange("p (s t) -> p s t", t=TPC),
                                fb, op=Alu.mult)
                            nc.tensor.matmul(psD[:], obf[:], pr[:],
                                             start=(mh == 0), stop=(mh == MH - 1))

                        # scores = dot / sqrt(s2_repr * s2_fut)
                        den = pb.tile([1, W], f32, tag="den", bufs=1)
                        s2fb = s2f.unsqueeze(1).to_broadcast([1, NS, TPC])
                        nc.vector.tensor_tensor(
                            den.rearrange("o (s t) -> o s t", t=TPC),
                            psS2.rearrange("o (s t) -> o s t", t=TPC),
                            s2fb, op=Alu.mult)
                        nc.scalar.activation(den[:], den[:], Act.Sqrt)
                        nc.vector.reciprocal(den[:], den[:])
                        nc.vector.tensor_mul(den[:], den[:], psD[:])
                        # token-major scatter: scd[t*A + a] = den[a-slot, t]
                        scdv = scd.ap().rearrange("(t a) -> a t", a=A)
                        a0 = (c * CH) // TPC
                        nc.sync.dma_start(
                            scdv[a0:a0 + NS, :],
                            den.rearrange("o (s t) -> o s t", t=TPC))

            # ================= Stage C: per-token loss ===================
            if "B" in stages and "C" not in stages:
                with tc.tile_pool(name="dummy2", bufs=1) as pdum:
                    dz = pdum.tile([1, 8], f32, tag="dz")
                    nc.vector.memset(dz[:], 0.0)
                    nc.vector.tensor_add(dz[0:1, 0:1], vals17[0:1, 0, 0:1],
                                         glob[0:1, 0, 0:1])
                    nc.sync.dma_start(outd.ap(), dz[:])
            if "C" in stages:
              with tc.tile_pool(name="stageC", bufs=1) as pc2, \
                   tc.tile_pool(name="psumC", bufs=1, space="PSUM") as psc:
                    # scores row -> token layout via DRAM bounce
                    scT = pc2.tile([128, TT, A], f32, tag="scT")
                    nc.sync.dma_start(
                        scT[:], scd.ap().rearrange("(q a) -> q a", a=A)
                        .rearrange("(t p) a -> p t a", p=128))

                    mask = pc2.tile([128, TT, A], f32, tag="mask")
                    nc.vector.memset(mask[:], 1.0)
                    for tt in range(TT):
                        nc.vector.tensor_scalar(mask[:, tt, 16:17], dup[:, tt:tt + 1],
                                                -1.0, 1.0, op0=Alu.mult, op1=Alu.add)
                    mb = pc2.tile([128, TT, A], f32, tag="mb")
                    nc.vector.tensor_scalar(mb[:], mask[:], 1e9, -1e9,
                                            op0=Alu.mult, op1=Alu.add)
                    st = pc2.tile([128, TT, A], f32, tag="st")
                    nc.vector.tensor_mul(st[:], scT[:], mask[:])
                    nc.vector.tensor_add(st[:], st[:], mb[:])
                    mx = pc2.tile([128, TT], f32, tag="mx")
                    nc.vector.tensor_reduce(mx[:], st[:], axis=Ax.X, op=Alu.max)
                    negmx = pc2.tile([128, TT], f32, tag="negmx")
                    nc.vector.tensor_scalar(negmx[:], mx[:], -1.0, None, op0=Alu.mult)
                    ee = pc2.tile([128, TT, A], f32, tag="ee")
                    for tt in range(TT):
                        nc.scalar.activation(ee[:, tt, :], st[:, tt, :], Act.Exp,
                                             bias=negmx[:, tt:tt + 1])
                    Z = pc2.tile([128, TT], f32, tag="Z")
                    nc.vector.tensor_reduce(Z[:], ee[:], axis=Ax.X, op=Alu.add)

                    # lp = logit - lse ; ew = sum(e * lp)
                    lp = pc2.tile([128, TT, A], f32, tag="lp")
                    for tt in range(TT):
                        nc.vector.tensor_scalar(lp[:, tt, :], vals17[:, tt, :],
                                                lse[:, tt:tt + 1], None,
                                                op0=Alu.subtract)
                    ew = pc2.tile([128, TT], f32, tag="ew")
                    elp = pc2.tile([128, TT, A], f32, tag="elp")
                    nc.vector.tensor_mul(elp[:], ee[:], lp[:])
                    nc.vector.tensor_reduce(ew[:], elp[:], axis=Ax.X,
                                            op=Alu.add)

                    # valid = (label != -100) & future_valid
                    valid = pc2.tile([128, TT], f32, tag="valid")
                    nc.vector.tensor_scalar(valid[:], labsb[:], -100.0, None,
                                            op0=Alu.not_equal)
                    nc.vector.tensor_mul(valid[:], valid[:], fvsb[:])
                    # denom = 17 - dup ; per-token term = ew/Z/denom*valid
                    den2 = pc2.tile([128, TT], f32, tag="den2")
                    nc.vector.tensor_scalar(den2[:], dup[:], -1.0, float(A),
                                            op0=Alu.mult, op1=Alu.add)
                    nc.vector.tensor_mul(den2[:], den2[:], Z[:])
                    rec = pc2.tile([128, TT], f32, tag="rec")
                    nc.vector.reciprocal(rec[:], den2[:])
                    nc.vector.tensor_mul(rec[:], rec[:], ew[:])
                    nc.vector.tensor_mul(rec[:], rec[:], valid[:])

                    rv = pc2.tile([128, 2], f32, tag="rv")
                    nc.vector.tensor_reduce(rv[:, 0:1], rec[:], axis=Ax.X, op=Alu.add)
                    nc.vector.tensor_reduce(rv[:, 1:2], valid[:], axis=Ax.X, op=Alu.add)
                    ocol2 = pc2.tile([128, 1], f32, tag="ocol2")
                    nc.sync.dma_start(ocol2[:], onescol.ap())
                    psF = psc.tile([1, 2], f32, tag="psF")
                    nc.tensor.matmul(psF[:], ocol2[:], rv[:])
                    outsb = pc2.tile([1, 8], f32, tag="outsb")
                    nc.vector.memset(outsb[:], 0.0)
                    nc.vector.tensor_copy(outsb[:, 0:2], psF[:])
                    nc.sync.dma_start(outd.ap(), outsb[:])

    nc.compile()
    return nc


# ---------------------------------------------------------------------------
# host-side input prep
# ---------------------------------------------------------------------------

def prep_core_inputs(core, logits, hidden_states, labels, future_summaries,
                     future_valid, embed_bf, W1bf, W2bf, w2r_l, b1_l, b2_l,
                     g_l, b_l, sumb2_v, V=32000, TPC=256, H=1024, GRP=256):
    TT = TPC // 128
    K16 = 16
    A = 17
    NW = A * TPC // 16
    G = V // GRP
    lo = core * TPC
    lg = np.ascontiguousarray(logits[lo:lo + TPC])
    hT = np.ascontiguousarray(hidden_states[lo:lo + TPC].T.astype(BF16))
    fT = np.ascontiguousarray(future_summaries[lo:lo + TPC].T.astype(BF16))
    lab = labels[lo:lo + TPC].astype(np.int64)
    fv = future_valid[lo:lo + TPC]

    labf = np.ascontiguousarray(lab.reshape(TT, 128).T.astype(np.float32))
    fvf = np.ascontiguousarray(fv.reshape(TT, 128).T.astype(np.float32))
    tok = (np.arange(TT)[None, :] * 128 + np.arange(128)[:, None])
    lgidx = (tok.astype(np.uint64) * V + lab.reshape(TT, 128).T).astype(np.uint32)
    lgidx = np.ascontiguousarray(lgidx)
    lwrap = np.zeros((16, NW), np.int16)
    # slot 16 (label): wrapped[q, TPC + u] = label[u*16 + q]
    lwrap[:, TPC:TPC + TPC // 16] = lab.reshape(TPC // 16, 16).T.astype(np.int16)
    jio = np.broadcast_to(np.arange(K16, dtype=np.float32)[None, :],
                          (128, K16)).copy()
    pfG = (np.arange(128, dtype=np.float32) * G).reshape(128, 1)
    thr = np.broadcast_to(
        (np.arange(1, 16, dtype=np.float32) * GRP)[None, :], (128, 15)).copy()

    return dict(
        logits=lg, hiddenT=hT, futT=fT, embed=embed_bf, W1=W1bf, W2=W2bf,
        w2r=w2r_l, b1d=b1_l, b2d=b2_l, gd=g_l, bd=b_l,
        sumb2=np.array([[sumb2_v]], np.float32),
        labf=labf, fvf=fvf, lgidx=lgidx, lwrap=lwrap, jio=jio, pfG=pfG,
        thr=thr,
        onesbf=np.ones((128, 1), BF16),
        onesrow=np.ones((1, 128), np.float32),
        onescol=np.ones((128, 1), np.float32),
    )


def prep_all_inputs(inputs, V=32000, TPC=256, H=1024, GRP=256, n_cores=8):
    logits = np.asarray(inputs["logits"], np.float32).reshape(-1, V)
    hidden = np.asarray(inputs["hidden_states"], np.float32).reshape(-1, H)
    labels = np.asarray(inputs["labels"]).reshape(-1)
    future = np.asarray(inputs["future_summaries"], np.float32).reshape(-1, H)
    fvalid = np.asarray(inputs["future_valid"]).reshape(-1)
    embed_bf = np.ascontiguousarray(np.asarray(inputs["embed_weight"]).astype(BF16))
    W1bf = np.ascontiguousarray(np.asarray(inputs["W1"]).astype(BF16))
    W2bf = np.ascontiguousarray(np.asarray(inputs["W2"]).astype(BF16))
    INNER = W1bf.shape[0]
    MI = INNER // 128
    MH = H // 128
    w2r = W2bf.astype(np.float32).sum(axis=1)
    w2r_l = np.ascontiguousarray(w2r.reshape(MI, 128).T.astype(BF16))
    b1 = np.asarray(inputs["b1"], np.float32)
    b2 = np.asarray(inputs["b2"], np.float32)
    ln_g = np.asarray(inputs["ln_g"], np.float32)
    ln_b = np.asarray(inputs["ln_b"], np.float32)
    b1_l = np.ascontiguousarray(b1.reshape(MI, 128).T)
    b2_l = np.ascontiguousarray(b2.reshape(MH, 128).T)
    g_l = np.ascontiguousarray(ln_g.reshape(MH, 128).T)
    b_l = np.ascontiguousarray(ln_b.reshape(MH, 128).T)
    sumb2_v = float(b2.sum()) / H

    return [
        prep_core_inputs(c, logits, hidden, labels, future, fvalid, embed_bf,
                         W1bf, W2bf, w2r_l, b1_l, b2_l, g_l, b_l, sumb2_v,
                         V=V, TPC=TPC, H=H, GRP=GRP)
        for c in range(n_cores)
    ]


_NC_CACHE = {}


def kernel(**inputs):
    from concourse.bass_utils import run_bass_kernel_spmd

    key = "main"
    if key not in _NC_CACHE:
        _NC_CACHE[key] = build_program()
    nc = _NC_CACHE[key]

    in_maps = prep_all_inputs(inputs)
    res = run_bass_kernel_spmd(nc, in_maps, core_ids=list(range(8)))
    num = 0.0
    cnt = 0.0
    for r in res.results:
        o = r["out"]
        num += float(o[0, 0])
        cnt += float(o[0, 1])
    loss = -num / max(cnt, 1.0)
    return np.array(loss, dtype=np.float32)

